# revision 49
# baseline (speedup 1.0000x reference)
# kernel.py — Mixtral layer (attention + top-2 MoE) on 8 TRN2 NeuronCores.
# Tensor-parallel: attention heads + MoE ffn dim sharded across cores,
# AllReduce (bf16) after o_proj and after MoE w2 (row-parallel).
# MoE GEMMs run in fp8 (e4m3, DoubleRow perf mode = 2x PE rate); weights are
# pre-scaled x32 on host to sit in e4m3's normal range, compensated on device.
# Self-contained: hardcodes all shapes; host pre-shards/transposes/casts.
import numpy as np
import ml_dtypes

BF16 = ml_dtypes.bfloat16
FP8 = ml_dtypes.float8_e4m3

HID = 1024
NH = 16
NKV = 4
HD = 64
E = 8
FFN = 2048
EPS = 1e-5
THETA = 10000.0
NCORES = 8
FS = FFN // NCORES  # 256 ffn rows per core per expert

WSCALE = 32.0       # fp8 weight pre-scale (w13, w2)
GSCALE = 8.0        # fp8 lift for g = silu(a)*b*wc
GFOLD = GSCALE / (WSCALE * WSCALE)  # routing-weight fold: t2 = WSCALE^2 * g


# ----------------------------------------------------------------------------
# Device program
# ----------------------------------------------------------------------------
def build_program(S, mock_cc=False):
    import concourse.bass as bass
    import concourse.mybir as mybir
    import concourse.tile as tile
    from concourse import bacc
    from concourse.bass import ts, ds

    dt = mybir.dt
    f32 = dt.float32
    bf16 = dt.bfloat16
    fp8 = dt.float8e4
    AF = mybir.ActivationFunctionType
    OP = mybir.AluOpType
    DR = mybir.MatmulPerfMode.DoubleRow

    NS = S // 512          # 512-wide token slices
    NT = S // 128          # 128-wide token tiles
    HC = HID // 128        # 8 hidden chunks
    HP = HC // 2           # 4 chunk pairs (fp8 DoubleRow)

    nc = bacc.Bacc("TRN2", target_bir_lowering=False, debug=False,
                   num_devices=NCORES)

    # ---- I/O ----
    xT_in = nc.dram_tensor("xT", [HID, S], bf16, kind="ExternalInput").ap()
    cos2_in = nc.dram_tensor("cos2", [128, S], bf16, kind="ExternalInput").ap()
    sin2_in = nc.dram_tensor("sin2", [128, S], bf16, kind="ExternalInput").ap()
    wqT_in = nc.dram_tensor("wqT", [HID, 128], bf16, kind="ExternalInput").ap()
    wkT_in = nc.dram_tensor("wkT", [HID, 64], bf16, kind="ExternalInput").ap()
    wvT_in = nc.dram_tensor("wvT", [HID, 64], bf16, kind="ExternalInput").ap()
    woT_in = nc.dram_tensor("woT", [128, HID], bf16, kind="ExternalInput").ap()
    gateT_in = nc.dram_tensor("gateT", [HID, E], bf16, kind="ExternalInput").ap()
    w1sT_in = nc.dram_tensor("w1sT", [E, HID, FS], fp8, kind="ExternalInput").ap()
    w3sT_in = nc.dram_tensor("w3sT", [E, HID, FS], fp8, kind="ExternalInput").ap()
    w2sT_in = nc.dram_tensor("w2sT", [E, FS, HID], fp8, kind="ExternalInput").ap()
    out_ext = nc.dram_tensor("out", [HID, S], bf16, kind="ExternalOutput").ap()

    xT_re = xT_in.rearrange("(c p) t -> p c t", p=128)
    outT_re = out_ext.rearrange("(c p) t -> p c t", p=128)

    RG = [list(range(NCORES))]

    with tile.TileContext(nc) as tc:
        # ---------- pool stack ----------
        cpool = tc.alloc_tile_pool(name="consts", bufs=1)
        dram = tc.alloc_tile_pool(name="dram", bufs=1, space="DRAM")
        mh = tc.alloc_tile_pool(name="mh", bufs=1)  # h8 + wfT (live into MoE)

        # constants
        ones128_bf = cpool.tile([128, 1], bf16)
        nc.vector.memset(ones128_bf, 1.0)
        onesr_f32 = cpool.tile([1, 128], f32)
        nc.vector.memset(onesr_f32, 1.0)
        ones8_bf = cpool.tile([8, 128], bf16)
        nc.vector.memset(ones8_bf, 1.0)
        # epack: rows 0 and 32 select head0/head1 reciprocal rows
        epack = cpool.tile([64, 128], f32)
        nc.vector.memset(epack, 0.0)
        nc.vector.memset(epack[0:1, 0:64], 1.0)
        nc.vector.memset(epack[32:33, 64:128], 1.0)
        # identity (f32) for PE transposes
        ident = cpool.tile([128, 128], f32)
        nc.vector.memset(ident, 1.0)
        nc.gpsimd.affine_select(ident, ident, pattern=[[1, 128]],
                                compare_op=OP.is_equal, fill=0.0,
                                base=0, channel_multiplier=-1)

        # attention weights
        wq_sb = cpool.tile([128, HC, 128], bf16)
        nc.sync.dma_start(wq_sb, wqT_in.rearrange("(c p) m -> p c m", p=128))
        wk_sb = cpool.tile([128, HC, 64], bf16)
        nc.sync.dma_start(wk_sb, wkT_in.rearrange("(c p) m -> p c m", p=128))
        wv_sb = cpool.tile([128, HC, 64], bf16)
        nc.sync.dma_start(wv_sb, wvT_in.rearrange("(c p) m -> p c m", p=128))
        wo_sb = cpool.tile([128, HID], bf16)
        nc.sync.dma_start(wo_sb, woT_in)
        gate_sb = cpool.tile([128, HC, E], bf16)
        nc.sync.dma_start(gate_sb, gateT_in.rearrange("(c p) m -> p c m", p=128))

        # DRAM bounce buffers for collectives (chunked per 512-token slice)
        delta_dram = dram.tile([NS, HID, 512], bf16)
        y_dram = dram.tile([NS, HID, 512], bf16)
        delta_ars = [dram.tile([HID, 512], bf16, addr_space="Shared",
                               name=f"dar{si}") for si in range(NS)]
        y_ars = [dram.tile([HID, 512], bf16, addr_space="Shared",
                           name=f"yar{si}") for si in range(NS)]
        s_dram = dram.tile([1, S], f32)        # rms2 per-token scale (natural)
        wf_dram = dram.tile([1, E, S], bf16)
        dar_res = [t.rearrange("(c p) t -> p c t", p=128) for t in delta_ars]
        yar_res = [t.rearrange("(c p) t -> p c t", p=128) for t in y_ars]

        wtp_sb = mh.tile([E, NT, 128], bf16)   # routing weights, [expert, token]
        # resident fp8 MoE weights (DMAs issued during attention)
        w1_sb = mh.tile([128, E, HC, FS], fp8)
        w3_sb = mh.tile([128, E, HC, FS], fp8)
        w2_sb = mh.tile([128, E, 2, HID], fp8)

        # ---------- x resident in SBUF (T layout) ----------
        xpool = tc.alloc_tile_pool(name="xpool", bufs=1)
        xsb = xpool.tile([128, HC, S], bf16)
        for c in range(HC):
            nc.sync.dma_start(xsb[:, c, :], xT_re[:, c, :])

        # ---------- attention (ln1 folded into cos/sin + v scale) ----------
        attnpool = tc.alloc_tile_pool(name="attnpool", bufs=1)

        cos_sb = attnpool.tile([128, S], bf16)
        nc.sync.dma_start(cos_sb, cos2_in)
        sin_sb = attnpool.tile([128, S], bf16)
        nc.sync.dma_start(sin_sb, sin2_in)
        # prefetch MoE weights (completes while attention computes)
        for e in range(E):
            nc.sync.dma_start(w1_sb[:, e], w1sT_in[e].rearrange("(c p) f -> p c f", p=128))
            nc.sync.dma_start(w3_sb[:, e], w3sT_in[e].rearrange("(c p) f -> p c f", p=128))
        nc.sync.dma_start(w2_sb, w2sT_in.rearrange("e (ct p) m -> p e ct m", p=128))
        cosS = attnpool.tile([128, S], bf16)
        sinS = attnpool.tile([128, S], bf16)
        s1_dram = dram.tile([1, S], f32)
        s_nat1 = attnpool.tile([128, NT], f32)

        # rms1 stats: per-token rsqrt(mean(x^2)+eps) -> sccast rows + s_nat1
        with tc.tile_pool(name="rms1", bufs=2) as rp, \
             tc.tile_pool(name="rms1p", bufs=1, space="PSUM") as pp:
            ss = []
            for si in range(NS):
                t = pp.tile([1, 512], f32, tag="ss", bufs=NS, name=f"ss{si}")
                ss.append(t)
            for c in range(HC):
                sq = rp.tile([128, S], bf16, tag="sq", bufs=2, name="sq")
                nc.scalar.activation(sq, xsb[:, c, :], AF.Square)
                for si in range(NS):
                    nc.tensor.matmul(ss[si], ones128_bf, sq[:, ds(512 * si, 512)],
                                     start=(c == 0), stop=(c == HC - 1))
            for si in range(NS):
                sl = ds(512 * si, 512)
                u = rp.tile([1, 512], f32, tag="u", name="u")
                nc.vector.tensor_scalar(u, ss[si], 1.0 / HID, EPS, OP.mult, OP.add)
                r = rp.tile([1, 512], f32, tag="r", name="r")
                nc.vector.reciprocal(r, u)
                sc = rp.tile([1, 512], f32, tag="sc", name="sc")
                nc.scalar.activation(sc, r, AF.Sqrt)
                nc.sync.dma_start(s1_dram[0:1, sl], sc)
                scc = pp.tile([128, 512], f32, tag="sccast", bufs=NS,
                              name=f"sccast{si}")
                nc.tensor.matmul(scc, onesr_f32, sc)
                nc.vector.tensor_tensor(cosS[:, sl], cos_sb[:, sl], scc, OP.mult)
                nc.vector.tensor_tensor(sinS[:, sl], sin_sb[:, sl], scc, OP.mult)
            nc.sync.dma_start(
                s_nat1, s1_dram.rearrange("o (i p) -> p (o i)", p=128))

        qT_sb = attnpool.tile([64, 2, S], bf16)
        kT_sb = attnpool.tile([64, S], bf16)
        v_sb = attnpool.tile([128, NT, 65], bf16)
        nc.vector.memset(v_sb[:, :, 64:65], 1.0)

        def rope(dsts, src_ps, si, nrows):
            # src_ps: psum [nrows, 512] (nrows = 128 for q(2 heads), 64 for k)
            # dsts: list of per-64-row-group dst APs [64, 512]
            with tc.tile_pool(name="rope", bufs=2) as rpp:
                sl = ds(512 * si, 512)
                rot = rpp.tile([128, 512], bf16, tag="rot", name="rot")
                for h in range(nrows // 64):
                    b = 64 * h
                    nc.vector.tensor_scalar(rot[b:b + 32, :], src_ps[b + 32:b + 64, :],
                                            -1.0, None, OP.mult)
                    nc.vector.tensor_copy(rot[b + 32:b + 64, :], src_ps[b:b + 32, :])
                t1 = rpp.tile([128, 512], bf16, tag="t1", name="t1")
                nc.vector.tensor_tensor(t1[:nrows, :], src_ps, cosS[:nrows, sl], OP.mult)
                t2 = rpp.tile([128, 512], bf16, tag="t2", name="t2")
                nc.vector.tensor_tensor(t2[:nrows, :], rot[:nrows, :], sinS[:nrows, sl], OP.mult)
                for h, dst in enumerate(dsts):
                    b = 64 * h
                    nc.vector.tensor_tensor(dst, t1[b:b + 64, :], t2[b:b + 64, :], OP.add)

        with tc.tile_pool(name="qkvp", bufs=1, space="PSUM") as qp:
            # q: [64, 2, S] and k: [64, S]
            for si in range(NS):
                sl = ds(512 * si, 512)
                pq = qp.tile([128, 512], f32, tag="pqk", bufs=3, name=f"pq{si}")
                for c in range(HC):
                    nc.tensor.matmul(pq, wq_sb[:, c, :], xsb[:, c, sl],
                                     start=(c == 0), stop=(c == HC - 1))
                rope([qT_sb[:, 0, sl], qT_sb[:, 1, sl]], pq, si, 128)
                pk = qp.tile([128, 512], f32, tag="pqk", bufs=3, name=f"pk{si}")
                for c in range(HC):
                    nc.tensor.matmul(pk[:64, :], wk_sb[:, c, :], xsb[:, c, sl],
                                     start=(c == 0), stop=(c == HC - 1))
                rope([kT_sb[:, sl]], pk[:64, :], si, 64)
            # v natural: [S, 64] as [128, NT, 65] (col 64 = ones for row-sums)
            for i in range(NT):
                pv = qp.tile([128, 64], f32, tag="pv", bufs=2, name="pv")
                for c in range(HC):
                    nc.tensor.matmul(pv, xsb[:, c, ts(i, 128)], wv_sb[:, c, :],
                                     start=(c == 0), stop=(c == HC - 1))
                nc.vector.tensor_scalar(v_sb[:, i, 0:64], pv, s_nat1[:, i:i + 1],
                                        None, OP.mult)

        # attention: scores transposed [k, q]; exp without max-subtract
        with tc.tile_pool(name="atsb", bufs=2) as asb, \
             tc.tile_pool(name="atps", bufs=1, space="PSUM") as aps:
            for si in range(NS):
                sl = ds(512 * si, 512)
                attn_ps = [aps.tile([65, 512], f32, tag="attn", bufs=2, name=f"attn{h}")
                           for h in range(2)]
                njt = 4 * si + 4
                for j in range(njt):
                    # both q heads share one GQA k head: scores land in the
                    # two bank-halves of one tile; exp+mask run batched
                    st = aps.tile([128, 2, 512], f32, tag="st", bufs=2, name="st")
                    for h in range(2):
                        nc.tensor.matmul(st[:, h, :], kT_sb[:, ts(j, 128)],
                                         qT_sb[:, h, sl])
                    ex = asb.tile([128, 2, 512], bf16, tag="ex", bufs=4, name="ex")
                    nc.scalar.activation(ex, st, AF.Exp)
                    if j >= 4 * si:
                        nc.gpsimd.affine_select(
                            ex, ex, pattern=[[0, 2], [1, 512]],
                            compare_op=OP.is_ge, fill=0.0,
                            base=512 * si - 128 * j, channel_multiplier=-1)
                    for h in range(2):
                        nc.tensor.matmul(attn_ps[h], v_sb[:, j, :], ex[:, h, :],
                                         start=(j == 0), stop=(j == njt - 1))
                # normalize by 1/l  (l = row 64 of attn_ps)
                rp_sb = asb.tile([64, 512], f32, tag="rp", name="rp_sb")
                nc.vector.memset(rp_sb, 0.0)
                nc.vector.reciprocal(rp_sb[0:1, :], attn_ps[0][64:65, :])
                nc.vector.reciprocal(rp_sb[32:33, :], attn_ps[1][64:65, :])
                rc_ps = aps.tile([128, 512], f32, tag="rc", bufs=2, name="rc_ps")
                nc.tensor.matmul(rc_ps, epack, rp_sb)
                rc_sb = asb.tile([128, 512], f32, tag="rcsb", name="rc_sb")
                nc.scalar.copy(rc_sb, rc_ps)
                at_sb = asb.tile([128, 512], bf16, tag="atsb", name="at_sb")
                nc.vector.tensor_tensor(at_sb[0:64, :], attn_ps[0][0:64, :],
                                        rc_sb[0:64, :], OP.mult)
                nc.vector.tensor_tensor(at_sb[64:128, :], attn_ps[1][0:64, :],
                                        rc_sb[64:128, :], OP.mult)
                # delta = woT.T @ attn
                for m in range(HC):
                    dps = aps.tile([128, 512], f32, tag="rc", bufs=2, name="dps")
                    nc.tensor.matmul(dps, wo_sb[:, ts(m, 128)], at_sb)
                    dsb = asb.tile([128, 512], bf16, tag="dsb", name="dsb")
                    nc.vector.tensor_copy(dsb, dps)
                    nc.sync.dma_start(delta_dram[si, ts(m, 128), :], dsb)
                # AR1 for this slice; overlaps the rest of attention if the
                # collective doesn't occupy the queue
                if mock_cc:
                    nc.gpsimd.dma_start(delta_ars[si], delta_dram[si])
                else:
                    nc.gpsimd.collective_compute(
                        "AllReduce", OP.add, replica_groups=RG,
                        ins=[delta_dram[si].opt()],
                        outs=[delta_ars[si].opt()])
        attnpool.release()

        # ---------- per-slice pipeline: rms2+gate -> MoE A/B -> AR2 -> out ----
        # All PSUM flows through one 8-bank pool of [128,512]f32 tiles.
        wf_re = wf_dram.rearrange("o e (i t) -> (o e) i t", t=128)
        s_nat_re = s_dram.rearrange("o (i p) -> p (o i)", p=128)
        with tc.tile_pool(name="x2p", bufs=3) as x2p, \
             tc.tile_pool(name="msb", bufs=2) as msb, \
             tc.tile_pool(name="fin", bufs=2) as fin, \
             tc.tile_pool(name="univ", bufs=1, space="PSUM") as ups:

            def pt(nm):
                return ups.tile([128, 512], f32, tag="u", bufs=8, name=nm)

            def emit_final(fsi, x2_f):
                # out stays in T layout (bf16); host transposes + casts
                fl = ds(512 * fsi, 512)
                for c in range(HC):
                    ya = fin.tile([128, 512], bf16, tag="ya", name="ya")
                    nc.scalar.dma_start(ya, yar_res[fsi][:, c, :])
                    sm2 = fin.tile([128, 512], bf16, tag="sm2", name="sm2")
                    nc.gpsimd.tensor_tensor(sm2, ya, x2_f[:, c, :], OP.add)
                    nc.sync.dma_start(outT_re[:, c, fl], sm2)

            x2_hist = {}
            for si in range(NS):
                sl = ds(512 * si, 512)
                # --- rms2: x2 = x + delta (resident per slice), stats, h8 ---
                x2 = x2p.tile([128, HC, 512], bf16, tag="x2", bufs=3, name="x2")
                x2_hist[si] = x2
                h8 = x2p.tile([128, HC, 512], fp8, tag="h8", bufs=2, name="h8")
                ssp = pt(f"ss{si}")
                for c in range(HC):
                    dr = msb.tile([128, 512], bf16, tag="dr", bufs=2, name="dr")
                    nc.sync.dma_start(dr, dar_res[si][:, c, :])
                    nc.vector.tensor_tensor(x2[:, c, :], xsb[:, c, sl], dr, OP.add)
                    sq = msb.tile([128, 512], bf16, tag="sq2", bufs=2, name="sq2")
                    nc.scalar.activation(sq, x2[:, c, :], AF.Square)
                    nc.tensor.matmul(ssp[0:1, :], ones128_bf, sq,
                                     start=(c == 0), stop=(c == HC - 1))
                u = msb.tile([1, 512], f32, tag="u2", name="u2")
                nc.vector.tensor_scalar(u, ssp[0:1, :], 1.0 / HID, EPS,
                                        OP.mult, OP.add)
                r = msb.tile([1, 512], f32, tag="r2", name="r2")
                nc.vector.reciprocal(r, u)
                sc = msb.tile([1, 512], f32, tag="sc2", name="sc2")
                nc.scalar.activation(sc, r, AF.Sqrt)
                nc.sync.dma_start(s_dram[0:1, sl], sc)
                s_nat2 = x2p.tile([128, 4], f32, tag="snat2", bufs=2, name="snat2")
                nc.sync.dma_start(s_nat2, s_nat_re[:, 4 * si:4 * si + 4])
                sccp = pt(f"scc{si}")
                nc.tensor.matmul(sccp, onesr_f32, sc)
                for c in range(HC):
                    nc.vector.tensor_tensor(h8[:, c, :], x2[:, c, :], sccp, OP.mult)
                # --- gate (unscaled logits; token scale inside sigmoid) ---
                for ii in range(4):
                    i = 4 * si + ii
                    lgp = pt(f"lg{i}")
                    lg = lgp[:, 0:E]
                    for c in range(HC):
                        nc.tensor.matmul(lg, x2[:, c, ts(ii, 128)], gate_sb[:, c, :],
                                         start=(c == 0), stop=(c == HC - 1))
                    top = msb.tile([128, 8], f32, tag="top", name="top")
                    nc.vector.max(out=top, in_=lg)
                    dd = msb.tile([128, 1], f32, tag="dd", name="dd")
                    nc.vector.tensor_sub(dd, top[:, 0:1], top[:, 1:2])
                    w1t = msb.tile([128, 1], f32, tag="w1t", name="w1t")
                    nc.scalar.activation(w1t, dd, AF.Sigmoid,
                                         scale=s_nat2[:, ii:ii + 1])
                    w2t = msb.tile([128, 1], f32, tag="w2t", name="w2t")
                    nc.vector.tensor_scalar(w2t, w1t, -GFOLD, GFOLD, OP.mult, OP.add)
                    nc.vector.tensor_scalar(w1t, w1t, GFOLD, None, OP.mult)
                    eq1 = msb.tile([128, E], f32, tag="eq1", name="eq1")
                    nc.vector.tensor_scalar(eq1, lg, top[:, 0:1], None, OP.is_equal)
                    eq2 = msb.tile([128, E], f32, tag="eq2", name="eq2")
                    nc.vector.tensor_scalar(eq2, lg, top[:, 1:2], None, OP.is_equal)
                    wf1 = msb.tile([128, E], f32, tag="wf1", name="wf1")
                    nc.vector.tensor_scalar(wf1, eq1, w1t[:, 0:1], None, OP.mult)
                    wfull = msb.tile([128, E], f32, tag="wfull", name="wfull")
                    nc.vector.scalar_tensor_tensor(wfull, eq2, w2t[:, 0:1], wf1,
                                                   OP.mult, OP.add)
                    wtpp = pt(f"wtp{i}")
                    nc.tensor.transpose(wtpp[0:E, 0:128], wfull, ident)
                    nc.scalar.copy(wtp_sb[:, i, :], wtpp[0:E, 0:128])
                # repack routing rows into partition 0 via DRAM bounce
                nc.sync.dma_start(wf_re[:, 4 * si:4 * si + 4, :],
                                  wtp_sb[:, 4 * si:4 * si + 4, :])
                wfT_si = msb.tile([1, E, 512], bf16, tag="wfr", bufs=1, name="wfr")
                nc.sync.dma_start(wfT_si, wf_dram[0:1, :, sl])
                # --- MoE phase A (fp8 DoubleRow) ---
                g_si = x2p.tile([128, 2 * E, 512], fp8, tag="g", bufs=2, name="g")
                for e in range(E):
                    p13 = []
                    for w_sb in (w1_sb, w3_sb):
                        for mt in range(2):
                            p = pt(f"p13_{e}_{mt}")
                            for k in range(HP):
                                nc.tensor.matmul(
                                    p, w_sb[:, e, 2 * k:2 * k + 2, ts(mt, 128)],
                                    h8[:, 2 * k:2 * k + 2, :],
                                    start=(k == 0), stop=(k == HP - 1),
                                    perf_mode=DR)
                            p13.append(p)
                    wc_ps = pt(f"wc{e}")
                    nc.tensor.matmul(wc_ps, ones8_bf[0:1, :], wfT_si[0:1, e, :])
                    wc_sb = msb.tile([128, 512], bf16, tag="wcsb", name="wc_sb")
                    nc.scalar.copy(wc_sb, wc_ps)
                    for mt in range(2):
                        # p13 = 32a ; sigmoid(a) = Sigmoid(p13/32)
                        s1 = msb.tile([128, 512], bf16, tag="s1m", name="s1")
                        nc.scalar.activation(s1, p13[mt], AF.Sigmoid,
                                             scale=1.0 / WSCALE)
                        t1 = msb.tile([128, 512], bf16, tag="t1m", name="t1")
                        nc.vector.tensor_tensor(t1, s1, p13[mt], OP.mult)
                        t2 = msb.tile([128, 512], bf16, tag="t2m", name="t2")
                        nc.vector.tensor_tensor(t2, t1, p13[2 + mt], OP.mult)
                        # t2 = 1024*g ; wc carries w*(8/1024) -> g' = 8*g*w
                        nc.gpsimd.tensor_tensor(g_si[:, 2 * e + mt, :], t2, wc_sb,
                                                OP.mult)
                # --- MoE phase B (fp8 DoubleRow, accumulate over experts) ---
                for m in range(HC):
                    y_ps = pt(f"y{m}")
                    for e in range(E):
                        nc.tensor.matmul(y_ps, w2_sb[:, e, :, ts(m, 128)],
                                         g_si[:, 2 * e:2 * e + 2, :],
                                         start=(e == 0), stop=(e == E - 1),
                                         perf_mode=DR)
                    y_sb = msb.tile([128, 512], bf16, tag="ysb", name="y_sb")
                    # y_ps = 256 * y_true
                    nc.vector.tensor_scalar(y_sb, y_ps, 1.0 / (WSCALE * GSCALE),
                                            None, OP.mult)
                    nc.sync.dma_start(y_dram[si, ts(m, 128), :], y_sb)
                # --- AR2 for this slice ---
                if mock_cc:
                    nc.gpsimd.dma_start(y_ars[si], y_dram[si])
                else:
                    nc.gpsimd.collective_compute(
                        "AllReduce", OP.add, replica_groups=RG,
                        ins=[y_dram[si].opt()],
                        outs=[y_ars[si].opt()])
                # --- final assembly, one slice behind ---
                if si >= 1:
                    emit_final(si - 1, x2_hist.pop(si - 1))
            emit_final(NS - 1, x2_hist.pop(NS - 1))

        xpool.release()
        mh.release()
        dram.release()
        cpool.release()
    nc.compile()
    return nc


# ----------------------------------------------------------------------------
# Host-side sharding / prep
# ----------------------------------------------------------------------------
def make_in_maps(x, ln1_w, ln2_w, wqkv, wo, gate_w, w13, w2):
    S = x.shape[1]
    x2d = np.asarray(x, np.float32).reshape(S, HID)
    ln1 = np.asarray(ln1_w, np.float32)
    ln2 = np.asarray(ln2_w, np.float32)
    wqkv = np.asarray(wqkv, np.float32)
    wo = np.asarray(wo, np.float32)
    gate_w = np.asarray(gate_w, np.float32)
    w13 = np.asarray(w13, np.float32)
    w2 = np.asarray(w2, np.float32)

    # rope tables
    inv_freq = 1.0 / (THETA ** (np.arange(0, HD, 2, dtype=np.float32) / HD))
    freqs = np.arange(S, dtype=np.float32)[:, None] * inv_freq[None, :]
    emb = np.concatenate([freqs, freqs], axis=-1)  # [S, 64]
    cosT = np.cos(emb).T  # [64, S]
    sinT = np.sin(emb).T
    cos2 = np.ascontiguousarray(np.concatenate([cosT, cosT], 0)).astype(BF16)
    sin2 = np.ascontiguousarray(np.concatenate([sinT, sinT], 0)).astype(BF16)

    xT = np.ascontiguousarray(x2d.T).astype(BF16)      # [HID, S]

    Wq = wqkv[:NH * HD]
    Wk = wqkv[NH * HD:(NH + NKV) * HD]
    Wv = wqkv[(NH + NKV) * HD:]
    gateT = np.ascontiguousarray((gate_w * ln2[None, :]).T).astype(BF16)

    in_maps = []
    for c in range(NCORES):
        g = c // 2
        wq_c = Wq[2 * c * HD:(2 * c + 2) * HD] * ln1[None, :] * (HD ** -0.5)
        wk_c = Wk[g * HD:(g + 1) * HD] * ln1[None, :]
        wv_c = Wv[g * HD:(g + 1) * HD] * ln1[None, :]
        woT_c = wo[:, 2 * c * HD:(2 * c + 2) * HD].T  # [128, HID]
        w1sT = np.stack([
            (w13[e, c * FS:(c + 1) * FS, :] * ln2[None, :] * WSCALE).T
            for e in range(E)])
        w3sT = np.stack([
            (w13[e, FFN + c * FS:FFN + (c + 1) * FS, :] * ln2[None, :] * WSCALE).T
            for e in range(E)])
        w2sT = np.stack([(w2[e][:, c * FS:(c + 1) * FS] * WSCALE).T
                         for e in range(E)])
        in_maps.append({
            "xT": xT, "cos2": cos2, "sin2": sin2,
            "wqT": np.ascontiguousarray(wq_c.T).astype(BF16),
            "wkT": np.ascontiguousarray(wk_c.T).astype(BF16),
            "wvT": np.ascontiguousarray(wv_c.T).astype(BF16),
            "woT": np.ascontiguousarray(woT_c).astype(BF16),
            "gateT": gateT,
            "w1sT": np.ascontiguousarray(w1sT).astype(FP8),
            "w3sT": np.ascontiguousarray(w3sT).astype(FP8),
            "w2sT": np.ascontiguousarray(w2sT).astype(FP8),
        })
    return in_maps


_CACHED = {}


def kernel(x, ln1_w, ln2_w, wqkv, wo, gate_w, w13, w2):
    from concourse import bass_utils
    S = x.shape[1]
    in_maps = make_in_maps(x, ln1_w, ln2_w, wqkv, wo, gate_w, w13, w2)
    if S not in _CACHED:
        _CACHED[S] = build_program(S)
    nc = _CACHED[S]
    res = bass_utils.run_bass_kernel_spmd(nc, in_maps, core_ids=list(range(NCORES)))
    out = res.results[0]["out"]  # [HID, S] bf16 (T layout)
    return np.ascontiguousarray(
        np.asarray(out, np.float32).T).reshape(1, S, HID)


if __name__ == "__main__":
    import reference
    inputs = {k: np.asarray(v) for k, v in reference.setup_inputs().items()}
    expected = np.asarray(reference.reference(**{k: v for k, v in inputs.items()}))
    actual = kernel(**inputs)
    err = np.linalg.norm(actual - expected) / np.linalg.norm(expected)
    print("Relative error:", err)


# revision 50
# speedup vs baseline: 1.0058x; 1.0058x over previous
# kernel.py — Mixtral layer (attention + top-2 MoE) on 8 TRN2 NeuronCores.
# Tensor-parallel: attention heads + MoE ffn dim sharded across cores,
# AllReduce (bf16) after o_proj and after MoE w2 (row-parallel).
# MoE GEMMs run in fp8 (e4m3, DoubleRow perf mode = 2x PE rate); weights are
# pre-scaled x32 on host to sit in e4m3's normal range, compensated on device.
# Self-contained: hardcodes all shapes; host pre-shards/transposes/casts.
import numpy as np
import ml_dtypes

BF16 = ml_dtypes.bfloat16
FP8 = ml_dtypes.float8_e4m3

HID = 1024
NH = 16
NKV = 4
HD = 64
E = 8
FFN = 2048
EPS = 1e-5
THETA = 10000.0
NCORES = 8
FS = FFN // NCORES  # 256 ffn rows per core per expert

WSCALE = 32.0       # fp8 weight pre-scale (w13, w2)
GSCALE = 8.0        # fp8 lift for g = silu(a)*b*wc
GFOLD = GSCALE / (WSCALE * WSCALE)  # routing-weight fold: t2 = WSCALE^2 * g


# ----------------------------------------------------------------------------
# Device program
# ----------------------------------------------------------------------------
def build_program(S, mock_cc=False):
    import concourse.bass as bass
    import concourse.mybir as mybir
    import concourse.tile as tile
    from concourse import bacc
    from concourse.bass import ts, ds

    dt = mybir.dt
    f32 = dt.float32
    bf16 = dt.bfloat16
    fp8 = dt.float8e4
    AF = mybir.ActivationFunctionType
    OP = mybir.AluOpType
    DR = mybir.MatmulPerfMode.DoubleRow

    NS = S // 512          # 512-wide token slices
    NT = S // 128          # 128-wide token tiles
    HC = HID // 128        # 8 hidden chunks
    HP = HC // 2           # 4 chunk pairs (fp8 DoubleRow)

    nc = bacc.Bacc("TRN2", target_bir_lowering=False, debug=False,
                   num_devices=NCORES)

    # ---- I/O ----
    xT_in = nc.dram_tensor("xT", [HID, S], bf16, kind="ExternalInput").ap()
    cos2_in = nc.dram_tensor("cos2", [128, S], bf16, kind="ExternalInput").ap()
    sin2_in = nc.dram_tensor("sin2", [128, S], bf16, kind="ExternalInput").ap()
    wqT_in = nc.dram_tensor("wqT", [HID, 128], bf16, kind="ExternalInput").ap()
    wkT_in = nc.dram_tensor("wkT", [HID, 64], bf16, kind="ExternalInput").ap()
    wvT_in = nc.dram_tensor("wvT", [HID, 64], bf16, kind="ExternalInput").ap()
    woT_in = nc.dram_tensor("woT", [128, HID], bf16, kind="ExternalInput").ap()
    gateT_in = nc.dram_tensor("gateT", [HID, E], bf16, kind="ExternalInput").ap()
    w1sT_in = nc.dram_tensor("w1sT", [E, HID, FS], fp8, kind="ExternalInput").ap()
    w3sT_in = nc.dram_tensor("w3sT", [E, HID, FS], fp8, kind="ExternalInput").ap()
    w2sT_in = nc.dram_tensor("w2sT", [E, FS, HID], fp8, kind="ExternalInput").ap()
    out_ext = nc.dram_tensor("out", [HID, S], bf16, kind="ExternalOutput").ap()

    xT_re = xT_in.rearrange("(c p) t -> p c t", p=128)
    outT_re = out_ext.rearrange("(c p) t -> p c t", p=128)

    RG = [list(range(NCORES))]

    with tile.TileContext(nc) as tc:
        # ---------- pool stack ----------
        cpool = tc.alloc_tile_pool(name="consts", bufs=1)
        dram = tc.alloc_tile_pool(name="dram", bufs=1, space="DRAM")
        mh = tc.alloc_tile_pool(name="mh", bufs=1)  # h8 + wfT (live into MoE)

        # constants
        ones128_bf = cpool.tile([128, 1], bf16)
        nc.vector.memset(ones128_bf, 1.0)
        onesr_f32 = cpool.tile([1, 128], f32)
        nc.vector.memset(onesr_f32, 1.0)
        ones8_bf = cpool.tile([8, 128], bf16)
        nc.vector.memset(ones8_bf, 1.0)
        # epack: rows 0 and 32 select head0/head1 reciprocal rows
        epack = cpool.tile([64, 128], f32)
        nc.vector.memset(epack, 0.0)
        nc.vector.memset(epack[0:1, 0:64], 1.0)
        nc.vector.memset(epack[32:33, 64:128], 1.0)
        # identity (f32) for PE transposes
        ident = cpool.tile([128, 128], f32)
        nc.vector.memset(ident, 1.0)
        nc.gpsimd.affine_select(ident, ident, pattern=[[1, 128]],
                                compare_op=OP.is_equal, fill=0.0,
                                base=0, channel_multiplier=-1)

        # attention weights
        wq_sb = cpool.tile([128, HC, 128], bf16)
        nc.sync.dma_start(wq_sb, wqT_in.rearrange("(c p) m -> p c m", p=128))
        wk_sb = cpool.tile([128, HC, 64], bf16)
        nc.sync.dma_start(wk_sb, wkT_in.rearrange("(c p) m -> p c m", p=128))
        wv_sb = cpool.tile([128, HC, 64], bf16)
        nc.sync.dma_start(wv_sb, wvT_in.rearrange("(c p) m -> p c m", p=128))
        wo_sb = cpool.tile([128, HID], bf16)
        nc.sync.dma_start(wo_sb, woT_in)
        gate_sb = cpool.tile([128, HC, E], bf16)
        nc.sync.dma_start(gate_sb, gateT_in.rearrange("(c p) m -> p c m", p=128))

        # DRAM bounce buffers for collectives (chunked per 512-token slice)
        delta_dram = dram.tile([NS, HID, 512], bf16)
        y_dram = dram.tile([NS, HID, 512], bf16)
        delta_ars = [dram.tile([HID, 512], bf16, addr_space="Shared",
                               name=f"dar{si}") for si in range(NS)]
        y_ars = [dram.tile([HID, 512], bf16, addr_space="Shared",
                           name=f"yar{si}") for si in range(NS)]
        s_dram = dram.tile([1, S], f32)        # rms2 per-token scale (natural)
        wf_dram = dram.tile([1, E, S], bf16)
        dar_res = [t.rearrange("(c p) t -> p c t", p=128) for t in delta_ars]
        yar_res = [t.rearrange("(c p) t -> p c t", p=128) for t in y_ars]

        wtp_sb = mh.tile([E, NT, 128], bf16)   # routing weights, [expert, token]
        # resident fp8 MoE weights (DMAs issued during attention)
        w1_sb = mh.tile([128, E, HC, FS], fp8)
        w3_sb = mh.tile([128, E, HC, FS], fp8)
        w2_sb = mh.tile([128, E, 2, HID], fp8)

        # ---------- x resident in SBUF (T layout) ----------
        xpool = tc.alloc_tile_pool(name="xpool", bufs=1)
        xsb = xpool.tile([128, HC, S], bf16)
        for c in range(HC):
            nc.sync.dma_start(xsb[:, c, :], xT_re[:, c, :])

        # ---------- attention (ln1 folded into cos/sin + v scale) ----------
        attnpool = tc.alloc_tile_pool(name="attnpool", bufs=1)

        cos_sb = attnpool.tile([128, S], bf16)
        nc.sync.dma_start(cos_sb, cos2_in)
        sin_sb = attnpool.tile([128, S], bf16)
        nc.sync.dma_start(sin_sb, sin2_in)
        cosS = attnpool.tile([128, S], bf16)
        sinS = attnpool.tile([128, S], bf16)
        s1_dram = dram.tile([1, S], f32)
        s_nat1 = attnpool.tile([128, NT], f32)

        # rms1 stats: per-token rsqrt(mean(x^2)+eps) -> sccast rows + s_nat1
        with tc.tile_pool(name="rms1", bufs=2) as rp, \
             tc.tile_pool(name="rms1p", bufs=1, space="PSUM") as pp:
            ss = []
            for si in range(NS):
                t = pp.tile([1, 512], f32, tag="ss", bufs=NS, name=f"ss{si}")
                ss.append(t)
            for c in range(HC):
                sq = rp.tile([128, S], bf16, tag="sq", bufs=2, name="sq")
                nc.scalar.activation(sq, xsb[:, c, :], AF.Square)
                for si in range(NS):
                    nc.tensor.matmul(ss[si], ones128_bf, sq[:, ds(512 * si, 512)],
                                     start=(c == 0), stop=(c == HC - 1))
            for si in range(NS):
                sl = ds(512 * si, 512)
                u = rp.tile([1, 512], f32, tag="u", name="u")
                nc.vector.tensor_scalar(u, ss[si], 1.0 / HID, EPS, OP.mult, OP.add)
                r = rp.tile([1, 512], f32, tag="r", name="r")
                nc.vector.reciprocal(r, u)
                sc = rp.tile([1, 512], f32, tag="sc", name="sc")
                nc.scalar.activation(sc, r, AF.Sqrt)
                nc.sync.dma_start(s1_dram[0:1, sl], sc)
                scc = pp.tile([128, 512], f32, tag="sccast", bufs=NS,
                              name=f"sccast{si}")
                nc.tensor.matmul(scc, onesr_f32, sc)
                nc.vector.tensor_tensor(cosS[:, sl], cos_sb[:, sl], scc, OP.mult)
                nc.vector.tensor_tensor(sinS[:, sl], sin_sb[:, sl], scc, OP.mult)
            nc.sync.dma_start(
                s_nat1, s1_dram.rearrange("o (i p) -> p (o i)", p=128))

        # prefetch MoE weights (behind the attention-critical SP traffic;
        # completes while attention computes)
        for e in range(E):
            nc.sync.dma_start(w1_sb[:, e], w1sT_in[e].rearrange("(c p) f -> p c f", p=128))
            nc.sync.dma_start(w3_sb[:, e], w3sT_in[e].rearrange("(c p) f -> p c f", p=128))
        nc.sync.dma_start(w2_sb, w2sT_in.rearrange("e (ct p) m -> p e ct m", p=128))

        qT_sb = attnpool.tile([64, 2, S], bf16)
        kT_sb = attnpool.tile([64, S], bf16)
        v_sb = attnpool.tile([128, NT, 65], bf16)
        nc.vector.memset(v_sb[:, :, 64:65], 1.0)

        def rope(dsts, src_ps, si, nrows):
            # src_ps: psum [nrows, 512] (nrows = 128 for q(2 heads), 64 for k)
            # dsts: list of per-64-row-group dst APs [64, 512]
            with tc.tile_pool(name="rope", bufs=2) as rpp:
                sl = ds(512 * si, 512)
                rot = rpp.tile([128, 512], bf16, tag="rot", name="rot")
                for h in range(nrows // 64):
                    b = 64 * h
                    nc.vector.tensor_scalar(rot[b:b + 32, :], src_ps[b + 32:b + 64, :],
                                            -1.0, None, OP.mult)
                    nc.vector.tensor_copy(rot[b + 32:b + 64, :], src_ps[b:b + 32, :])
                t1 = rpp.tile([128, 512], bf16, tag="t1", name="t1")
                nc.vector.tensor_tensor(t1[:nrows, :], src_ps, cosS[:nrows, sl], OP.mult)
                t2 = rpp.tile([128, 512], bf16, tag="t2", name="t2")
                nc.vector.tensor_tensor(t2[:nrows, :], rot[:nrows, :], sinS[:nrows, sl], OP.mult)
                for h, dst in enumerate(dsts):
                    b = 64 * h
                    nc.vector.tensor_tensor(dst, t1[b:b + 64, :], t2[b:b + 64, :], OP.add)

        with tc.tile_pool(name="qkvp", bufs=1, space="PSUM") as qp:
            # q: [64, 2, S] and k: [64, S]
            for si in range(NS):
                sl = ds(512 * si, 512)
                pq = qp.tile([128, 512], f32, tag="pqk", bufs=3, name=f"pq{si}")
                for c in range(HC):
                    nc.tensor.matmul(pq, wq_sb[:, c, :], xsb[:, c, sl],
                                     start=(c == 0), stop=(c == HC - 1))
                rope([qT_sb[:, 0, sl], qT_sb[:, 1, sl]], pq, si, 128)
                pk = qp.tile([128, 512], f32, tag="pqk", bufs=3, name=f"pk{si}")
                for c in range(HC):
                    nc.tensor.matmul(pk[:64, :], wk_sb[:, c, :], xsb[:, c, sl],
                                     start=(c == 0), stop=(c == HC - 1))
                rope([kT_sb[:, sl]], pk[:64, :], si, 64)
            # v natural: [S, 64] as [128, NT, 65] (col 64 = ones for row-sums)
            for i in range(NT):
                pv = qp.tile([128, 64], f32, tag="pv", bufs=2, name="pv")
                for c in range(HC):
                    nc.tensor.matmul(pv, xsb[:, c, ts(i, 128)], wv_sb[:, c, :],
                                     start=(c == 0), stop=(c == HC - 1))
                nc.vector.tensor_scalar(v_sb[:, i, 0:64], pv, s_nat1[:, i:i + 1],
                                        None, OP.mult)

        # attention: scores transposed [k, q]; exp without max-subtract
        with tc.tile_pool(name="atsb", bufs=2) as asb, \
             tc.tile_pool(name="atps", bufs=1, space="PSUM") as aps:
            for si in range(NS):
                sl = ds(512 * si, 512)
                attn_ps = [aps.tile([65, 512], f32, tag="attn", bufs=2, name=f"attn{h}")
                           for h in range(2)]
                njt = 4 * si + 4
                for j in range(njt):
                    # both q heads share one GQA k head: scores land in the
                    # two bank-halves of one tile; exp+mask run batched
                    st = aps.tile([128, 2, 512], f32, tag="st", bufs=2, name="st")
                    for h in range(2):
                        nc.tensor.matmul(st[:, h, :], kT_sb[:, ts(j, 128)],
                                         qT_sb[:, h, sl])
                    ex = asb.tile([128, 2, 512], bf16, tag="ex", bufs=4, name="ex")
                    nc.scalar.activation(ex, st, AF.Exp)
                    if j >= 4 * si:
                        nc.gpsimd.affine_select(
                            ex, ex, pattern=[[0, 2], [1, 512]],
                            compare_op=OP.is_ge, fill=0.0,
                            base=512 * si - 128 * j, channel_multiplier=-1)
                    for h in range(2):
                        nc.tensor.matmul(attn_ps[h], v_sb[:, j, :], ex[:, h, :],
                                         start=(j == 0), stop=(j == njt - 1))
                # normalize by 1/l  (l = row 64 of attn_ps)
                rp_sb = asb.tile([64, 512], f32, tag="rp", name="rp_sb")
                nc.vector.memset(rp_sb, 0.0)
                nc.vector.reciprocal(rp_sb[0:1, :], attn_ps[0][64:65, :])
                nc.vector.reciprocal(rp_sb[32:33, :], attn_ps[1][64:65, :])
                rc_ps = aps.tile([128, 512], f32, tag="rc", bufs=2, name="rc_ps")
                nc.tensor.matmul(rc_ps, epack, rp_sb)
                rc_sb = asb.tile([128, 512], f32, tag="rcsb", name="rc_sb")
                nc.scalar.copy(rc_sb, rc_ps)
                at_sb = asb.tile([128, 512], bf16, tag="atsb", name="at_sb")
                nc.vector.tensor_tensor(at_sb[0:64, :], attn_ps[0][0:64, :],
                                        rc_sb[0:64, :], OP.mult)
                nc.vector.tensor_tensor(at_sb[64:128, :], attn_ps[1][0:64, :],
                                        rc_sb[64:128, :], OP.mult)
                # delta = woT.T @ attn
                for m in range(HC):
                    dps = aps.tile([128, 512], f32, tag="rc", bufs=2, name="dps")
                    nc.tensor.matmul(dps, wo_sb[:, ts(m, 128)], at_sb)
                    dsb = asb.tile([128, 512], bf16, tag="dsb", name="dsb")
                    nc.vector.tensor_copy(dsb, dps)
                    nc.sync.dma_start(delta_dram[si, ts(m, 128), :], dsb)
                # AR1 for this slice; overlaps the rest of attention if the
                # collective doesn't occupy the queue
                if mock_cc:
                    nc.gpsimd.dma_start(delta_ars[si], delta_dram[si])
                else:
                    nc.gpsimd.collective_compute(
                        "AllReduce", OP.add, replica_groups=RG,
                        ins=[delta_dram[si].opt()],
                        outs=[delta_ars[si].opt()])
        attnpool.release()

        # ---------- per-slice pipeline: rms2+gate -> MoE A/B -> AR2 -> out ----
        # All PSUM flows through one 8-bank pool of [128,512]f32 tiles.
        wf_re = wf_dram.rearrange("o e (i t) -> (o e) i t", t=128)
        s_nat_re = s_dram.rearrange("o (i p) -> p (o i)", p=128)
        with tc.tile_pool(name="x2p", bufs=3) as x2p, \
             tc.tile_pool(name="msb", bufs=2) as msb, \
             tc.tile_pool(name="fin", bufs=2) as fin, \
             tc.tile_pool(name="univ", bufs=1, space="PSUM") as ups:

            def pt(nm):
                return ups.tile([128, 512], f32, tag="u", bufs=8, name=nm)

            def emit_final(fsi, x2_f):
                # out stays in T layout (bf16); host transposes + casts
                fl = ds(512 * fsi, 512)
                for c in range(HC):
                    ya = fin.tile([128, 512], bf16, tag="ya", name="ya")
                    nc.scalar.dma_start(ya, yar_res[fsi][:, c, :])
                    sm2 = fin.tile([128, 512], bf16, tag="sm2", name="sm2")
                    nc.gpsimd.tensor_tensor(sm2, ya, x2_f[:, c, :], OP.add)
                    nc.sync.dma_start(outT_re[:, c, fl], sm2)

            x2_hist = {}
            for si in range(NS):
                sl = ds(512 * si, 512)
                # --- rms2: x2 = x + delta (resident per slice), stats, h8 ---
                x2 = x2p.tile([128, HC, 512], bf16, tag="x2", bufs=3, name="x2")
                x2_hist[si] = x2
                h8 = x2p.tile([128, HC, 512], fp8, tag="h8", bufs=2, name="h8")
                ssp = pt(f"ss{si}")
                for c in range(HC):
                    dr = msb.tile([128, 512], bf16, tag="dr", bufs=2, name="dr")
                    nc.sync.dma_start(dr, dar_res[si][:, c, :])
                    nc.vector.tensor_tensor(x2[:, c, :], xsb[:, c, sl], dr, OP.add)
                    sq = msb.tile([128, 512], bf16, tag="sq2", bufs=2, name="sq2")
                    nc.scalar.activation(sq, x2[:, c, :], AF.Square)
                    nc.tensor.matmul(ssp[0:1, :], ones128_bf, sq,
                                     start=(c == 0), stop=(c == HC - 1))
                u = msb.tile([1, 512], f32, tag="u2", name="u2")
                nc.vector.tensor_scalar(u, ssp[0:1, :], 1.0 / HID, EPS,
                                        OP.mult, OP.add)
                r = msb.tile([1, 512], f32, tag="r2", name="r2")
                nc.vector.reciprocal(r, u)
                sc = msb.tile([1, 512], f32, tag="sc2", name="sc2")
                nc.scalar.activation(sc, r, AF.Sqrt)
                nc.sync.dma_start(s_dram[0:1, sl], sc)
                s_nat2 = x2p.tile([128, 4], f32, tag="snat2", bufs=2, name="snat2")
                nc.sync.dma_start(s_nat2, s_nat_re[:, 4 * si:4 * si + 4])
                sccp = pt(f"scc{si}")
                nc.tensor.matmul(sccp, onesr_f32, sc)
                for c in range(HC):
                    nc.vector.tensor_tensor(h8[:, c, :], x2[:, c, :], sccp, OP.mult)
                # --- gate (unscaled logits; token scale inside sigmoid) ---
                for ii in range(4):
                    i = 4 * si + ii
                    lgp = pt(f"lg{i}")
                    lg = lgp[:, 0:E]
                    for c in range(HC):
                        nc.tensor.matmul(lg, x2[:, c, ts(ii, 128)], gate_sb[:, c, :],
                                         start=(c == 0), stop=(c == HC - 1))
                    top = msb.tile([128, 8], f32, tag="top", name="top")
                    nc.vector.max(out=top, in_=lg)
                    dd = msb.tile([128, 1], f32, tag="dd", name="dd")
                    nc.vector.tensor_sub(dd, top[:, 0:1], top[:, 1:2])
                    w1t = msb.tile([128, 1], f32, tag="w1t", name="w1t")
                    nc.scalar.activation(w1t, dd, AF.Sigmoid,
                                         scale=s_nat2[:, ii:ii + 1])
                    w2t = msb.tile([128, 1], f32, tag="w2t", name="w2t")
                    nc.vector.tensor_scalar(w2t, w1t, -GFOLD, GFOLD, OP.mult, OP.add)
                    nc.vector.tensor_scalar(w1t, w1t, GFOLD, None, OP.mult)
                    eq1 = msb.tile([128, E], f32, tag="eq1", name="eq1")
                    nc.vector.tensor_scalar(eq1, lg, top[:, 0:1], None, OP.is_equal)
                    eq2 = msb.tile([128, E], f32, tag="eq2", name="eq2")
                    nc.vector.tensor_scalar(eq2, lg, top[:, 1:2], None, OP.is_equal)
                    wf1 = msb.tile([128, E], f32, tag="wf1", name="wf1")
                    nc.vector.tensor_scalar(wf1, eq1, w1t[:, 0:1], None, OP.mult)
                    wfull = msb.tile([128, E], f32, tag="wfull", name="wfull")
                    nc.vector.scalar_tensor_tensor(wfull, eq2, w2t[:, 0:1], wf1,
                                                   OP.mult, OP.add)
                    wtpp = pt(f"wtp{i}")
                    nc.tensor.transpose(wtpp[0:E, 0:128], wfull, ident)
                    nc.scalar.copy(wtp_sb[:, i, :], wtpp[0:E, 0:128])
                # repack routing rows into partition 0 via DRAM bounce
                nc.sync.dma_start(wf_re[:, 4 * si:4 * si + 4, :],
                                  wtp_sb[:, 4 * si:4 * si + 4, :])
                wfT_si = msb.tile([1, E, 512], bf16, tag="wfr", bufs=1, name="wfr")
                nc.sync.dma_start(wfT_si, wf_dram[0:1, :, sl])
                # --- MoE phase A (fp8 DoubleRow) ---
                g_si = x2p.tile([128, 2 * E, 512], fp8, tag="g", bufs=2, name="g")
                for e in range(E):
                    p13 = []
                    for w_sb in (w1_sb, w3_sb):
                        for mt in range(2):
                            p = pt(f"p13_{e}_{mt}")
                            for k in range(HP):
                                nc.tensor.matmul(
                                    p, w_sb[:, e, 2 * k:2 * k + 2, ts(mt, 128)],
                                    h8[:, 2 * k:2 * k + 2, :],
                                    start=(k == 0), stop=(k == HP - 1),
                                    perf_mode=DR)
                            p13.append(p)
                    wc_ps = pt(f"wc{e}")
                    nc.tensor.matmul(wc_ps, ones8_bf[0:1, :], wfT_si[0:1, e, :])
                    wc_sb = msb.tile([128, 512], bf16, tag="wcsb", name="wc_sb")
                    nc.scalar.copy(wc_sb, wc_ps)
                    for mt in range(2):
                        # p13 = 32a ; sigmoid(a) = Sigmoid(p13/32)
                        s1 = msb.tile([128, 512], bf16, tag="s1m", name="s1")
                        nc.scalar.activation(s1, p13[mt], AF.Sigmoid,
                                             scale=1.0 / WSCALE)
                        t1 = msb.tile([128, 512], bf16, tag="t1m", name="t1")
                        nc.vector.tensor_tensor(t1, s1, p13[mt], OP.mult)
                        t2 = msb.tile([128, 512], bf16, tag="t2m", name="t2")
                        nc.vector.tensor_tensor(t2, t1, p13[2 + mt], OP.mult)
                        # t2 = 1024*g ; wc carries w*(8/1024) -> g' = 8*g*w
                        nc.gpsimd.tensor_tensor(g_si[:, 2 * e + mt, :], t2, wc_sb,
                                                OP.mult)
                # --- MoE phase B (fp8 DoubleRow, accumulate over experts) ---
                for m in range(HC):
                    y_ps = pt(f"y{m}")
                    for e in range(E):
                        nc.tensor.matmul(y_ps, w2_sb[:, e, :, ts(m, 128)],
                                         g_si[:, 2 * e:2 * e + 2, :],
                                         start=(e == 0), stop=(e == E - 1),
                                         perf_mode=DR)
                    y_sb = msb.tile([128, 512], bf16, tag="ysb", name="y_sb")
                    # y_ps = 256 * y_true
                    nc.vector.tensor_scalar(y_sb, y_ps, 1.0 / (WSCALE * GSCALE),
                                            None, OP.mult)
                    nc.sync.dma_start(y_dram[si, ts(m, 128), :], y_sb)
                # --- AR2 for this slice ---
                if mock_cc:
                    nc.gpsimd.dma_start(y_ars[si], y_dram[si])
                else:
                    nc.gpsimd.collective_compute(
                        "AllReduce", OP.add, replica_groups=RG,
                        ins=[y_dram[si].opt()],
                        outs=[y_ars[si].opt()])
                # --- final assembly, one slice behind ---
                if si >= 1:
                    emit_final(si - 1, x2_hist.pop(si - 1))
            emit_final(NS - 1, x2_hist.pop(NS - 1))

        xpool.release()
        mh.release()
        dram.release()
        cpool.release()
    nc.compile()
    return nc


# ----------------------------------------------------------------------------
# Host-side sharding / prep
# ----------------------------------------------------------------------------
def make_in_maps(x, ln1_w, ln2_w, wqkv, wo, gate_w, w13, w2):
    S = x.shape[1]
    x2d = np.asarray(x, np.float32).reshape(S, HID)
    ln1 = np.asarray(ln1_w, np.float32)
    ln2 = np.asarray(ln2_w, np.float32)
    wqkv = np.asarray(wqkv, np.float32)
    wo = np.asarray(wo, np.float32)
    gate_w = np.asarray(gate_w, np.float32)
    w13 = np.asarray(w13, np.float32)
    w2 = np.asarray(w2, np.float32)

    # rope tables
    inv_freq = 1.0 / (THETA ** (np.arange(0, HD, 2, dtype=np.float32) / HD))
    freqs = np.arange(S, dtype=np.float32)[:, None] * inv_freq[None, :]
    emb = np.concatenate([freqs, freqs], axis=-1)  # [S, 64]
    cosT = np.cos(emb).T  # [64, S]
    sinT = np.sin(emb).T
    cos2 = np.ascontiguousarray(np.concatenate([cosT, cosT], 0)).astype(BF16)
    sin2 = np.ascontiguousarray(np.concatenate([sinT, sinT], 0)).astype(BF16)

    xT = np.ascontiguousarray(x2d.T).astype(BF16)      # [HID, S]

    Wq = wqkv[:NH * HD]
    Wk = wqkv[NH * HD:(NH + NKV) * HD]
    Wv = wqkv[(NH + NKV) * HD:]
    gateT = np.ascontiguousarray((gate_w * ln2[None, :]).T).astype(BF16)

    in_maps = []
    for c in range(NCORES):
        g = c // 2
        wq_c = Wq[2 * c * HD:(2 * c + 2) * HD] * ln1[None, :] * (HD ** -0.5)
        wk_c = Wk[g * HD:(g + 1) * HD] * ln1[None, :]
        wv_c = Wv[g * HD:(g + 1) * HD] * ln1[None, :]
        woT_c = wo[:, 2 * c * HD:(2 * c + 2) * HD].T  # [128, HID]
        w1sT = np.stack([
            (w13[e, c * FS:(c + 1) * FS, :] * ln2[None, :] * WSCALE).T
            for e in range(E)])
        w3sT = np.stack([
            (w13[e, FFN + c * FS:FFN + (c + 1) * FS, :] * ln2[None, :] * WSCALE).T
            for e in range(E)])
        w2sT = np.stack([(w2[e][:, c * FS:(c + 1) * FS] * WSCALE).T
                         for e in range(E)])
        in_maps.append({
            "xT": xT, "cos2": cos2, "sin2": sin2,
            "wqT": np.ascontiguousarray(wq_c.T).astype(BF16),
            "wkT": np.ascontiguousarray(wk_c.T).astype(BF16),
            "wvT": np.ascontiguousarray(wv_c.T).astype(BF16),
            "woT": np.ascontiguousarray(woT_c).astype(BF16),
            "gateT": gateT,
            "w1sT": np.ascontiguousarray(w1sT).astype(FP8),
            "w3sT": np.ascontiguousarray(w3sT).astype(FP8),
            "w2sT": np.ascontiguousarray(w2sT).astype(FP8),
        })
    return in_maps


_CACHED = {}


def kernel(x, ln1_w, ln2_w, wqkv, wo, gate_w, w13, w2):
    from concourse import bass_utils
    S = x.shape[1]
    in_maps = make_in_maps(x, ln1_w, ln2_w, wqkv, wo, gate_w, w13, w2)
    if S not in _CACHED:
        _CACHED[S] = build_program(S)
    nc = _CACHED[S]
    res = bass_utils.run_bass_kernel_spmd(nc, in_maps, core_ids=list(range(NCORES)))
    out = res.results[0]["out"]  # [HID, S] bf16 (T layout)
    return np.ascontiguousarray(
        np.asarray(out, np.float32).T).reshape(1, S, HID)


if __name__ == "__main__":
    import reference
    inputs = {k: np.asarray(v) for k, v in reference.setup_inputs().items()}
    expected = np.asarray(reference.reference(**{k: v for k, v in inputs.items()}))
    actual = kernel(**inputs)
    err = np.linalg.norm(actual - expected) / np.linalg.norm(expected)
    print("Relative error:", err)
